# revision 23
# baseline (speedup 1.0000x reference)
"""Trainium2 Bass kernel for nn_AttentionBlock (B=8, S=2048, D=512, f32).

Strategy: data-parallel over batch — one batch element per NeuronCore (8 cores,
same NEFF, SPMD). Per core, the full attention block is computed with the
"transposed scores" layout so no on-chip transposes are needed.

Key algebraic trick (merged QK): scores = (x Wq^T)(x Wk^T)^T / sqrt(D)
= x A x^T with A = Wq^T Wk / sqrt(D) precomputed on the host. This removes
one full projection (the k-projection) from the device: the scores matmul
contracts qaT = A^T x^T directly against xt, which doubles as the k-side
stationary operand.

  host prep:  xt = x[b].T            [D, S]
              wa = Wq^T Wk / sqrt(D) [D, D]   (d rows, e cols)
              wv = Wv.T              [D, D]
  stage A:    qaT[e, s] = sum_d wa[d, e] * xt[d, s]    (PSUM accum over d)
              v[s, e]   = sum_d xt[d, s] * wv[d, e]    (interleaved per s-chunk)
  stage B:    sT[k, q] = sum_e xt[e, k] * qaT[e, q]    (scores, transposed)
              p[k, q]  = exp(sT)     -- no max subtraction: scores in [-10, 10]
              h[q]     = running per-partition sum of p (one DVE add per kb)
  stage C:    outT[e, q] = sum_k v[k, e] * p[k, q]
              outT *= 1/l  (GPSIMD partition all-reduce of h -> DVE reciprocal)
  host post:  out[b] = outT.T

All tensors feeding the PE are bf16: bf16 keeps the matmul streaming rate of
f32r (1 col/cycle) but halves LDWEIGHTS time (97 ns vs 224 ns measured), which
un-hides-from/hides-under the 213 ns moving-operand stream — per-MM rate drops
from 272 ns (f32r) to the 216 ns floor. It also halves input DMA bytes.
Accumulation is fp32 in PSUM throughout; measured end-to-end rel err ~7e-3
(gate 2e-2).

Emission order is tuned so the PE never waits: ~24 wide (256-col) bf16 warmup
matmuls ramp the HAM clock while inputs DMA in (1-col warmups do NOT trip the
HAM activity monitor — measured); stage A runs s-chunk-major so the first
matmuls only need wa's first column block + the first xt chunk; the
v-projection fills the gap between scores(qc=0) and PV(qc=0); the denominator
is a running DVE add behind each exp so 1/l is ready one PV group after the
scores finish, letting the normalize+DMA of each output block start as soon
as its PV group completes (small kernel tail).
"""

import math

import numpy as np

import concourse.bass_isa as bass_isa
import concourse.mybir as mybir
import concourse.tile as tile
from concourse import bacc
from concourse.bass_utils import run_bass_kernel_spmd

P = 128          # partitions
S = 2048         # sequence length
DM = 512         # d_model == d_attn == d_value
ND = DM // P     # 4  d-model chunks
NS = S // P      # 16 sequence blocks
QC = 512         # q-chunk width for fused score/PV stages
NQC = S // QC    # 4
NEC = DM // P    # 4  e-chunks of the output
N_WARMUP = 40    # wide PE warmup matmuls issued while input DMAs stream

F32 = mybir.dt.float32
F32R = mybir.dt.float32r
BF16 = mybir.dt.bfloat16

# 'bf16' (default): bf16 storage+matmuls.  'f32r': f32 storage, float32r matmuls.
MODE = "bf16"

_NC_CACHE = {}


def _build(mode):
    # tensors feeding the tensor engine carry the matmul dtype: the BIR
    # verifier requires fp32r matmul operands to be *produced* as float32r
    sb_dt = BF16 if mode == "bf16" else F32R
    # aux dtype for the l1 / broadcast matmul chain: bf16 runs those matmuls
    # single-pass; fp32 would be the 4-pass LOW/HIGH mode (~700 ns each)
    aux_dt = BF16 if mode == "bf16" else F32R
    nc = bacc.Bacc()

    xt_d = nc.dram_tensor("xt", [DM, S], sb_dt, kind="ExternalInput")
    wa_d = nc.dram_tensor("wa", [DM, DM], sb_dt, kind="ExternalInput")
    wv_d = nc.dram_tensor("wv", [DM, DM], sb_dt, kind="ExternalInput")
    # output travels bf16 (halves the out-DMA; host upcasts to f32 — adds
    # ~0.4% worst-case to a ~5e-3 rel err, far under the 2e-2 gate)
    out_dt = BF16 if mode == "bf16" else F32
    outT_d = nc.dram_tensor("outT", [DM, S], out_dt, kind="ExternalOutput")

    mm = nc.tensor.matmul

    # low-precision outputs on DVE ops trip the guard; actual matmul
    # accumulation stays in fp32 PSUM throughout.
    with nc.allow_low_precision(reason="bf16/fp32r operand rounding; PSUM accumulation is fp32"), \
         tile.TileContext(nc) as tc:
        with tc.tile_pool(name="consts", bufs=1) as consts:
            # persistent SBUF tensors (distinct tags so nothing shares slots).
            # xt/wa/wv pack all d-chunks into ONE tile so each input needs a
            # single DMA trigger (dma_start costs ~0.6us of serial issue time
            # on the Sync queue; per-chunk triggers delayed the first stage-A
            # matmul by several us and let the HAM clock re-throttle)
            wa_all = consts.tile([P, ND, DM], sb_dt, name="wa", tag="wa")
            wv_all = consts.tile([P, ND, DM], sb_dt, name="wv", tag="wv")
            xt_all = consts.tile([P, ND, S], sb_dt, name="xt", tag="xt")
            qt_sb = [consts.tile([P, S], sb_dt, name=f"qt{j}", tag=f"qt{j}") for j in range(ND)]
            v_sb = [consts.tile([P, DM], sb_dt, name=f"v{b}", tag=f"v{b}") for b in range(NS)]
            # fp32 source for memset (memset can't write f32r) and the exp
            # table preload; warm_src feeds the wide warmup matmuls
            warm_raw = consts.tile([P, 256], F32, name="warm_raw", tag="warm_raw")
            warm_src = consts.tile([P, 256], sb_dt, name="warm_src", tag="warm_src")
            nc.vector.memset(warm_raw, 1.0)
            nc.vector.tensor_copy(warm_src, warm_raw)
            # preload the ACT Exp table during stage A — otherwise the first
            # exp of the scores stage pays the ~1.3us table load inline
            exp_warm = consts.tile([P, 1], F32, name="exp_warm", tag="exp_warm")
            nc.scalar.activation(out=exp_warm, in_=warm_raw[:, 0:1],
                                 func=mybir.ActivationFunctionType.Exp)

            # input DMAs in first-use order, FIVE triggers total (Tile tracks
            # sub-tile ranges, so stage-A groups gate on exactly the ranges
            # they read): wa j0-columns + the first xt half gate stage A's
            # sc=0/1 groups; wv lands before the interleaved v-projection
            xt_r = xt_d.rearrange("(i p) s -> p i s", p=P)
            wa_r = wa_d.rearrange("(i p) e -> p i e", p=P)
            wv_r = wv_d.rearrange("(i p) e -> p i e", p=P)
            nc.sync.dma_start(out=wa_all[:, :, 0:P], in_=wa_r[:, :, 0:P])
            nc.sync.dma_start(out=xt_all[:, :, 0:2 * QC], in_=xt_r[:, :, 0:2 * QC])
            nc.sync.dma_start(out=wa_all[:, :, P:DM], in_=wa_r[:, :, P:DM])
            nc.sync.dma_start(out=wv_all, in_=wv_r)
            nc.sync.dma_start(out=xt_all[:, :, 2 * QC:S], in_=xt_r[:, :, 2 * QC:S])

            # ---- stage A: qa projection (s-chunk-major: the first groups
            # only need wa's first columns + the first xt chunk) --------------
            # psS/psM are opened while psA is still live so they get banks the
            # stage-A pool never touches and carry NO dependency on psA's
            # release (a pool release waits on ALL of the pool's accessors)
            from contextlib import ExitStack as _ExitStack
            with (
                tc.tile_pool(name="psS", bufs=5, space="PSUM") as psS,
            ):
                _psa_stack = _ExitStack()
                psA = _psa_stack.enter_context(tc.tile_pool(name="psA", bufs=3, space="PSUM"))
                # PE warmup: wide matmuls with no data deps keep the PE array
                # genuinely busy while inputs stream in, so the HAM clock gate
                # opens (2.4 GHz) before real matmuls start. 1-col matmuls do
                # not register as PE activity for the HAM — these must be wide.
                warm = psA.tile([P, 256], F32, name="warm", tag="psA")
                for w in range(N_WARMUP):
                    mm(warm, warm_src[:, 0:P], warm_src, start=True, stop=True)
                # The v-projection interleaves with the qa groups per s-chunk:
                # each 512-col xt chunk unlocks ~6.9us of matmuls (4 qa + 4 v
                # groups) against ~3.5us of DMA, so the PE rides out the xt
                # stream without stalling.  Copies alternate ACT/DVE: both are
                # idle here, and spreading them means the first exp of the
                # scores stage isn't queued behind a backlog of stage-A copies
                for sc in range(NQC):
                    for j in range(ND):
                        ps = psA.tile([P, QC], F32, name="psA", tag="psA")
                        for i in range(ND):
                            mm(ps, wa_all[:, i, j * P:(j + 1) * P],
                               xt_all[:, i, sc * QC:(sc + 1) * QC],
                               start=(i == 0), stop=(i == ND - 1))
                        if j % 2 == 0:
                            nc.scalar.copy(qt_sb[j][:, sc * QC:(sc + 1) * QC], ps)
                        else:
                            nc.vector.tensor_copy(qt_sb[j][:, sc * QC:(sc + 1) * QC], ps)
                    for b in range(4 * sc, 4 * sc + 4):
                        psv = psA.tile([P, DM], F32, name="psv", tag="psA")
                        for i in range(ND):
                            mm(psv, xt_all[:, i, b * P:(b + 1) * P], wv_all[:, i, :],
                               start=(i == 0), stop=(i == ND - 1))
                        if b % 2 == 0:
                            nc.scalar.copy(v_sb[b], psv)
                        else:
                            nc.vector.tensor_copy(v_sb[b], psv)
                _psa_stack.close()

                # ---- stages B+C: scores -> exp -> denominators -> PV ------
                with (
                    tc.tile_pool(name="ptp", bufs=1) as ptp,
                    tc.tile_pool(name="work", bufs=2) as work,
                    tc.tile_pool(name="outp", bufs=3) as outp,
                    tc.tile_pool(name="psO", bufs=3, space="PSUM") as psO,
                ):
                    for qc in range(NQC):
                        qs = slice(qc * QC, (qc + 1) * QC)
                        pt = ptp.tile([P, NS, QC], sb_dt, name="pt", tag="pt")
                        # running per-partition sum of p on the (otherwise
                        # idle) DVE: one [P, QC] add right behind each exp, so
                        # the combined sum lands ~0.7us after the last exp
                        h1 = work.tile([P, QC], F32, name="h1", tag="h1", bufs=1)
                        for kb in range(NS):
                            ps_s = psS.tile([P, QC], F32, name="ps_s", tag="ps_s")
                            for j in range(ND):
                                mm(ps_s, xt_all[:, j, kb * P:(kb + 1) * P], qt_sb[j][:, qs],
                                   start=(j == 0), stop=(j == ND - 1))
                            nc.scalar.activation(out=pt[:, kb, :], in_=ps_s,
                                                 func=mybir.ActivationFunctionType.Exp)
                            if kb == 0:
                                nc.vector.tensor_copy(h1, pt[:, 0, :])
                            else:
                                nc.vector.tensor_add(h1, h1, pt[:, kb, :])

                        # l -> 1/l entirely off the PE: a GPSIMD partition
                        # all-reduce fuses the cross-partition sum AND the
                        # broadcast (every partition gets l), then one DVE
                        # reciprocal yields the [P, QC] normalizer.  Both run
                        # on otherwise-idle engines during the first PV group.
                        l_bc = work.tile([P, QC], F32, name="l_bc", tag="l_bc", bufs=1)
                        r_bc = work.tile([P, QC], F32, name="r_bc", tag="r_bc")
                        nc.gpsimd.partition_all_reduce(l_bc, h1, 128, bass_isa.ReduceOp.add)
                        nc.vector.reciprocal(out=r_bc, in_=l_bc)

                        # PV: outT[e, q] = sum_k v[k, e] * p[k, q]; each output
                        # block is normalized + DMA'd as soon as its PV group
                        # completes, so the kernel tail is one block's norm+DMA
                        last = qc == NQC - 1
                        for ec in range(NEC):
                            if last and ec == NEC - 1:
                                # final output tile in two column halves (separate
                                # PSUM banks — a shared bank would serialize on the
                                # first half's norm read): the first half's
                                # norm+DMA overlap the second half's matmuls
                                for h in range(2):
                                    hs = slice(h * (QC // 2), (h + 1) * (QC // 2))
                                    ps_h = psO.tile([P, QC // 2], F32, name="ps_h", tag="ps_o")
                                    for kb in range(NS):
                                        mm(ps_h, v_sb[kb][:, ec * P:(ec + 1) * P],
                                           pt[:, kb, hs], start=(kb == 0), stop=(kb == NS - 1))
                                    out_h = outp.tile([P, QC // 2], out_dt, name="out_h", tag="out_h")
                                    nc.vector.tensor_mul(out_h, ps_h, r_bc[:, hs])
                                    nc.sync.dma_start(
                                        out=outT_d[ec * P:(ec + 1) * P,
                                                   qc * QC + h * (QC // 2):qc * QC + (h + 1) * (QC // 2)],
                                        in_=out_h)
                                continue
                            ps_o = psO.tile([P, QC], F32, name="ps_o", tag="ps_o")
                            for kb in range(NS):
                                mm(ps_o, v_sb[kb][:, ec * P:(ec + 1) * P], pt[:, kb, :],
                                   start=(kb == 0), stop=(kb == NS - 1))
                            out_sb = outp.tile([P, QC], out_dt, name="out_sb", tag="out_sb")
                            nc.vector.tensor_mul(out_sb, ps_o, r_bc)
                            nc.sync.dma_start(out=outT_d[ec * P:(ec + 1) * P, qs], in_=out_sb)

    nc.compile()
    return nc


def _get_nc(mode):
    if mode not in _NC_CACHE:
        _NC_CACHE[mode] = _build(mode)
    return _NC_CACHE[mode]


def _prep_in_maps(x, Wq, Wk, Wv, mode):
    if mode == "bf16":
        import ml_dtypes

        def cast(a):
            return np.ascontiguousarray(a).astype(ml_dtypes.bfloat16)
    else:
        def cast(a):
            return np.ascontiguousarray(a, dtype=np.float32)

    scale = 1.0 / math.sqrt(DM)
    # merged QK: scores = x (Wq^T Wk / sqrt(D)) x^T
    wa_h = cast((np.asarray(Wq, np.float64).T @ np.asarray(Wk, np.float64)
                 * scale).astype(np.float32))
    wv_h = cast(np.asarray(Wv, np.float32).T)
    x = np.asarray(x, np.float32)
    return [
        {"xt": cast(x[b].T), "wa": wa_h, "wv": wv_h}
        for b in range(x.shape[0])
    ]


def _run(in_maps, mode=None, **kw):
    mode = mode or MODE
    nc = _get_nc(mode)
    return run_bass_kernel_spmd(nc, in_maps, core_ids=list(range(len(in_maps))), **kw)


def kernel(x, Wq, Wk, Wv):
    in_maps = _prep_in_maps(x, Wq, Wk, Wv, MODE)
    res = _run(in_maps)
    out = np.stack([np.asarray(r["outT"]).astype(np.float32).T for r in res.results])
    return np.ascontiguousarray(out, dtype=np.float32)


# revision 25
# speedup vs baseline: 1.1945x; 1.1945x over previous
"""Trainium2 Bass kernel for nn_AttentionBlock (B=8, S=2048, D=512, f32).

Strategy: data-parallel over batch — one batch element per NeuronCore (8 cores,
same NEFF, SPMD). Per core, the full attention block is computed with the
"transposed scores" layout so no on-chip transposes are needed.

Key algebraic trick (merged QK): scores = (x Wq^T)(x Wk^T)^T / sqrt(D)
= x A x^T with A = Wq^T Wk / sqrt(D) precomputed on the host. This removes
one full projection (the k-projection) from the device: the scores matmul
contracts qaT = A^T x^T directly against xt, which doubles as the k-side
stationary operand.

  host prep:  xt = x[b].T            [D, S]
              wa = Wq^T Wk / sqrt(D) [D, D]   (d rows, e cols)
              wv = Wv.T              [D, D]
  stage A:    qaT[e, s] = sum_d wa[d, e] * xt[d, s]    (PSUM accum over d)
              v[s, e]   = sum_d xt[d, s] * wv[d, e]    (interleaved per s-chunk)
  stage B:    sT[k, q] = sum_e xt[e, k] * qaT[e, q]    (scores, transposed)
              p[k, q]  = exp(sT)     -- no max subtraction: scores in [-10, 10]
              h[q]     = running per-partition sum of p (one DVE add per kb)
  stage C:    outT[e, q] = sum_k v[k, e] * p[k, q]
              outT *= 1/l  (GPSIMD partition all-reduce of h -> DVE reciprocal)
  host post:  out[b] = outT.T

All tensors feeding the PE are bf16: bf16 keeps the matmul streaming rate of
f32r (1 col/cycle) but halves LDWEIGHTS time (97 ns vs 224 ns measured), which
un-hides-from/hides-under the 213 ns moving-operand stream — per-MM rate drops
from 272 ns (f32r) to the 216 ns floor. It also halves input DMA bytes.
Accumulation is fp32 in PSUM throughout; measured end-to-end rel err ~7e-3
(gate 2e-2).

Emission order is tuned so the PE never waits: ~24 wide (256-col) bf16 warmup
matmuls ramp the HAM clock while inputs DMA in (1-col warmups do NOT trip the
HAM activity monitor — measured); stage A runs s-chunk-major so the first
matmuls only need wa's first column block + the first xt chunk; the
v-projection fills the gap between scores(qc=0) and PV(qc=0); the denominator
is a running DVE add behind each exp so 1/l is ready one PV group after the
scores finish, letting the normalize+DMA of each output block start as soon
as its PV group completes (small kernel tail).
"""

import math

import numpy as np

import concourse.bass_isa as bass_isa
import concourse.mybir as mybir
import concourse.tile as tile
from concourse import bacc
from concourse.bass_utils import run_bass_kernel_spmd

P = 128          # partitions
S = 2048         # sequence length
DM = 512         # d_model == d_attn == d_value
ND = DM // P     # 4  d-model chunks
NS = S // P      # 16 sequence blocks
QC = 512         # q-chunk width for fused score/PV stages
NQC = S // QC    # 4
NEC = DM // P    # 4  e-chunks of the output
N_WARMUP = 40    # wide PE warmup matmuls issued while input DMAs stream

F32 = mybir.dt.float32
F32R = mybir.dt.float32r
BF16 = mybir.dt.bfloat16

# 'bf16' (default): bf16 storage+matmuls.  'f32r': f32 storage, float32r matmuls.
MODE = "bf16"

_NC_CACHE = {}


def _build(mode):
    # tensors feeding the tensor engine carry the matmul dtype: the BIR
    # verifier requires fp32r matmul operands to be *produced* as float32r
    sb_dt = BF16 if mode == "bf16" else F32R
    nc = bacc.Bacc()

    xt_d = nc.dram_tensor("xt", [DM, S], sb_dt, kind="ExternalInput")
    wa_d = nc.dram_tensor("wa", [DM, DM], sb_dt, kind="ExternalInput")
    wv_d = nc.dram_tensor("wv", [DM, DM], sb_dt, kind="ExternalInput")
    # output travels bf16 (halves the out-DMA; host upcasts to f32 — adds
    # ~0.4% worst-case to a ~5e-3 rel err, far under the 2e-2 gate)
    out_dt = BF16 if mode == "bf16" else F32
    outT_d = nc.dram_tensor("outT", [DM, S], out_dt, kind="ExternalOutput")

    mm = nc.tensor.matmul

    # low-precision outputs on DVE ops trip the guard; actual matmul
    # accumulation stays in fp32 PSUM throughout.
    with nc.allow_low_precision(reason="bf16/fp32r operand rounding; PSUM accumulation is fp32"), \
         tile.TileContext(nc) as tc:
        with tc.tile_pool(name="consts", bufs=1) as consts:
            # persistent SBUF tensors (distinct tags so nothing shares slots).
            # xt/wa/wv pack all d-chunks into ONE tile so each input needs a
            # single DMA trigger (dma_start costs ~0.6us of serial issue time
            # on the Sync queue; per-chunk triggers delayed the first stage-A
            # matmul by several us and let the HAM clock re-throttle)
            wa_all = consts.tile([P, ND, DM], sb_dt, name="wa", tag="wa")
            wv_all = consts.tile([P, ND, DM], sb_dt, name="wv", tag="wv")
            xt_all = consts.tile([P, ND, S], sb_dt, name="xt", tag="xt")
            qt_sb = [consts.tile([P, S], sb_dt, name=f"qt{j}", tag=f"qt{j}") for j in range(ND)]
            v_sb = [consts.tile([P, DM], sb_dt, name=f"v{b}", tag=f"v{b}") for b in range(NS)]
            # fp32 source for memset (memset can't write f32r) and the exp
            # table preload; warm_src feeds the wide warmup matmuls
            warm_raw = consts.tile([P, 256], F32, name="warm_raw", tag="warm_raw")
            warm_src = consts.tile([P, 256], sb_dt, name="warm_src", tag="warm_src")
            nc.vector.memset(warm_raw, 1.0)
            nc.vector.tensor_copy(warm_src, warm_raw)
            # preload the ACT Exp table during stage A — otherwise the first
            # exp of the scores stage pays the ~1.3us table load inline
            exp_warm = consts.tile([P, 1], F32, name="exp_warm", tag="exp_warm")
            nc.scalar.activation(out=exp_warm, in_=warm_raw[:, 0:1],
                                 func=mybir.ActivationFunctionType.Exp)

            # input DMAs in first-use order, FIVE triggers total (Tile tracks
            # sub-tile ranges, so stage-A groups gate on exactly the ranges
            # they read): wa j0-columns + the first xt half gate stage A's
            # sc=0/1 groups; wv lands before the interleaved v-projection
            xt_r = xt_d.rearrange("(i p) s -> p i s", p=P)
            wa_r = wa_d.rearrange("(i p) e -> p i e", p=P)
            wv_r = wv_d.rearrange("(i p) e -> p i e", p=P)
            nc.sync.dma_start(out=wa_all[:, :, 0:P], in_=wa_r[:, :, 0:P])
            # first xt half in two quarters: a matmul gates on the WHOLE dma
            # it reads from (completion sems are per-dma), so the sc=0 groups
            # start ~1.7us earlier gated on 640KB instead of 1.1MB
            nc.sync.dma_start(out=xt_all[:, :, 0:QC], in_=xt_r[:, :, 0:QC])
            nc.sync.dma_start(out=xt_all[:, :, QC:2 * QC], in_=xt_r[:, :, QC:2 * QC])
            nc.sync.dma_start(out=wa_all[:, :, P:DM], in_=wa_r[:, :, P:DM])
            nc.sync.dma_start(out=wv_all, in_=wv_r)
            nc.sync.dma_start(out=xt_all[:, :, 2 * QC:S], in_=xt_r[:, :, 2 * QC:S])

            # ---- stage A: qa projection (s-chunk-major: the first groups
            # only need wa's first columns + the first xt chunk) --------------
            # psS/psM are opened while psA is still live so they get banks the
            # stage-A pool never touches and carry NO dependency on psA's
            # release (a pool release waits on ALL of the pool's accessors)
            from contextlib import ExitStack as _ExitStack
            with (
                tc.tile_pool(name="psS", bufs=5, space="PSUM") as psS,
            ):
                _psa_stack = _ExitStack()
                psA = _psa_stack.enter_context(tc.tile_pool(name="psA", bufs=3, space="PSUM"))
                # PE warmup: wide matmuls with no data deps keep the PE array
                # genuinely busy while inputs stream in, so the HAM clock gate
                # opens (2.4 GHz) before real matmuls start. 1-col matmuls do
                # not register as PE activity for the HAM — these must be wide.
                warm = psA.tile([P, 256], F32, name="warm", tag="psA")
                for w in range(N_WARMUP):
                    mm(warm, warm_src[:, 0:P], warm_src, start=True, stop=True)
                # The v-projection interleaves with the qa groups per s-chunk:
                # each 512-col xt chunk unlocks ~6.9us of matmuls (4 qa + 4 v
                # groups) against ~3.5us of DMA, so the PE rides out the xt
                # stream without stalling.  Copies alternate ACT/DVE: both are
                # idle here, and spreading them means the first exp of the
                # scores stage isn't queued behind a backlog of stage-A copies
                for sc in range(NQC):
                    for j in range(ND):
                        ps = psA.tile([P, QC], F32, name="psA", tag="psA")
                        for i in range(ND):
                            mm(ps, wa_all[:, i, j * P:(j + 1) * P],
                               xt_all[:, i, sc * QC:(sc + 1) * QC],
                               start=(i == 0), stop=(i == ND - 1))
                        if j % 2 == 0:
                            nc.scalar.copy(qt_sb[j][:, sc * QC:(sc + 1) * QC], ps)
                        else:
                            nc.vector.tensor_copy(qt_sb[j][:, sc * QC:(sc + 1) * QC], ps)
                    for b in range(4 * sc, 4 * sc + 4):
                        psv = psA.tile([P, DM], F32, name="psv", tag="psA")
                        for i in range(ND):
                            mm(psv, xt_all[:, i, b * P:(b + 1) * P], wv_all[:, i, :],
                               start=(i == 0), stop=(i == ND - 1))
                        if b % 2 == 0:
                            nc.scalar.copy(v_sb[b], psv)
                        else:
                            nc.vector.tensor_copy(v_sb[b], psv)
                _psa_stack.close()

                # ---- stages B+C: scores -> exp -> denominators -> PV ------
                with (
                    tc.tile_pool(name="ptp", bufs=1) as ptp,
                    tc.tile_pool(name="work", bufs=2) as work,
                    tc.tile_pool(name="outp", bufs=3) as outp,
                    tc.tile_pool(name="psO", bufs=3, space="PSUM") as psO,
                ):
                    for qc in range(NQC):
                        qs = slice(qc * QC, (qc + 1) * QC)
                        pt = ptp.tile([P, NS, QC], sb_dt, name="pt", tag="pt")
                        # running per-partition sum of p on the (otherwise
                        # idle) DVE: one [P, QC] add right behind each exp, so
                        # the combined sum lands ~0.7us after the last exp
                        h1 = work.tile([P, QC], F32, name="h1", tag="h1", bufs=1)
                        for kb in range(NS):
                            ps_s = psS.tile([P, QC], F32, name="ps_s", tag="ps_s")
                            for j in range(ND):
                                mm(ps_s, xt_all[:, j, kb * P:(kb + 1) * P], qt_sb[j][:, qs],
                                   start=(j == 0), stop=(j == ND - 1))
                            nc.scalar.activation(out=pt[:, kb, :], in_=ps_s,
                                                 func=mybir.ActivationFunctionType.Exp)
                            if kb == 0:
                                nc.vector.tensor_copy(h1, pt[:, 0, :])
                            else:
                                nc.vector.tensor_add(h1, h1, pt[:, kb, :])

                        # l -> 1/l entirely off the PE: a GPSIMD partition
                        # all-reduce fuses the cross-partition sum AND the
                        # broadcast (every partition gets l), then one DVE
                        # reciprocal yields the [P, QC] normalizer.  Both run
                        # on otherwise-idle engines during the first PV group.
                        l_bc = work.tile([P, QC], F32, name="l_bc", tag="l_bc", bufs=1)
                        r_bc = work.tile([P, QC], F32, name="r_bc", tag="r_bc")
                        nc.gpsimd.partition_all_reduce(l_bc, h1, 128, bass_isa.ReduceOp.add)
                        nc.vector.reciprocal(out=r_bc, in_=l_bc)

                        # PV: outT[e, q] = sum_k v[k, e] * p[k, q]; each output
                        # block is normalized + DMA'd as soon as its PV group
                        # completes, so the kernel tail is one block's norm+DMA
                        last = qc == NQC - 1
                        for ec in range(NEC):
                            if last and ec == NEC - 1:
                                # final output tile in two column halves (separate
                                # PSUM banks — a shared bank would serialize on the
                                # first half's norm read): the first half's
                                # norm+DMA overlap the second half's matmuls
                                for h in range(2):
                                    hs = slice(h * (QC // 2), (h + 1) * (QC // 2))
                                    ps_h = psO.tile([P, QC // 2], F32, name="ps_h", tag="ps_o")
                                    for kb in range(NS):
                                        mm(ps_h, v_sb[kb][:, ec * P:(ec + 1) * P],
                                           pt[:, kb, hs], start=(kb == 0), stop=(kb == NS - 1))
                                    out_h = outp.tile([P, QC // 2], out_dt, name="out_h", tag="out_h")
                                    nc.vector.tensor_mul(out_h, ps_h, r_bc[:, hs])
                                    nc.sync.dma_start(
                                        out=outT_d[ec * P:(ec + 1) * P,
                                                   qc * QC + h * (QC // 2):qc * QC + (h + 1) * (QC // 2)],
                                        in_=out_h)
                                continue
                            ps_o = psO.tile([P, QC], F32, name="ps_o", tag="ps_o")
                            for kb in range(NS):
                                mm(ps_o, v_sb[kb][:, ec * P:(ec + 1) * P], pt[:, kb, :],
                                   start=(kb == 0), stop=(kb == NS - 1))
                            out_sb = outp.tile([P, QC], out_dt, name="out_sb", tag="out_sb")
                            nc.vector.tensor_mul(out_sb, ps_o, r_bc)
                            nc.sync.dma_start(out=outT_d[ec * P:(ec + 1) * P, qs], in_=out_sb)

    nc.compile()
    return nc


def _get_nc(mode):
    if mode not in _NC_CACHE:
        _NC_CACHE[mode] = _build(mode)
    return _NC_CACHE[mode]


def _prep_in_maps(x, Wq, Wk, Wv, mode):
    if mode == "bf16":
        import ml_dtypes

        def cast(a):
            return np.ascontiguousarray(a).astype(ml_dtypes.bfloat16)
    else:
        def cast(a):
            return np.ascontiguousarray(a, dtype=np.float32)

    scale = 1.0 / math.sqrt(DM)
    # merged QK: scores = x (Wq^T Wk / sqrt(D)) x^T
    wa_h = cast((np.asarray(Wq, np.float64).T @ np.asarray(Wk, np.float64)
                 * scale).astype(np.float32))
    wv_h = cast(np.asarray(Wv, np.float32).T)
    x = np.asarray(x, np.float32)
    return [
        {"xt": cast(x[b].T), "wa": wa_h, "wv": wv_h}
        for b in range(x.shape[0])
    ]


def _run(in_maps, mode=None, **kw):
    mode = mode or MODE
    nc = _get_nc(mode)
    return run_bass_kernel_spmd(nc, in_maps, core_ids=list(range(len(in_maps))), **kw)


def kernel(x, Wq, Wk, Wv):
    in_maps = _prep_in_maps(x, Wq, Wk, Wv, MODE)
    res = _run(in_maps)
    out = np.stack([np.asarray(r["outT"]).astype(np.float32).T for r in res.results])
    return np.ascontiguousarray(out, dtype=np.float32)
